# revision 1
# baseline (speedup 1.0000x reference)
"""Trainium2 Bass kernel for nn_MultiHeadAttention_20066087207431.

Reference computation (B=2, S=2048, NV=1024, H=16, DH=64):
    Q = Sq @ Wq_w.T + Wq_b ; K = Sk @ Wq_w.T + Wq_b ; V = Sv @ Wq_w.T + Wq_b
    per (batch, head):  qk = Q K^T / sqrt(DH) ;  Y = qk @ V
    X = softmax(Y, axis=-1)          # softmax AFTER the second matmul (quirk)
    out = X @ out_w.T + out_b

Key algebraic optimization: there is no softmax between the two attention
matmuls, so (Q K^T) V == Q (K^T V).  K^T V is only (64, 64) per head, which
collapses ~34 GFLOP of score math into ~1 GFLOP and removes the (S, S) score
matrices entirely.  The kernel is then memory-bound.

Sharding: 8 cores; core c handles batch b = c // 4 and a 512-token slice of S
(data-parallel over batch*sequence).  Each core computes local partial
M = K_loc^T V_loc / sqrt(DH) for all 16 heads; a 512 KB AllReduce within each
4-core batch group completes the sum over S.  Everything else (projections,
Y = Q M, softmax, output projection) is local to the core.  Weights are
replicated.

On-device layouts (SBUF is 128 partitions x free):
    Q and the output are kept transposed (feature on partition, token free);
    K/V are produced in natural layout (token on partition) so the M matmuls
    can contract over tokens.  The host pre-transposes the input slices.
M is stored block-diagonal ((128,1024), head pair per 128-col chunk, zeros
off-diagonal) so Y = Q M needs one 128x128 matmul per (token, feature) chunk.
"""

import os
import sys

import numpy as np

for _p in ("/opt/trn_rl_repo", "/root/.axon_site/_ro/trn_rl_repo"):
    if os.path.isdir(_p) and _p not in sys.path:
        sys.path.insert(0, _p)

import concourse.bass as bass  # noqa: E402
import concourse.mybir as mybir  # noqa: E402
import concourse.tile as tile  # noqa: E402
from concourse import bacc  # noqa: E402
from concourse.bass_utils import run_bass_kernel_spmd  # noqa: E402
from concourse.masks import make_identity  # noqa: E402

F32 = mybir.dt.float32
F32R = mybir.dt.float32r
AF = mybir.ActivationFunctionType
ALU = mybir.AluOpType

B, S, NV = 2, 2048, 1024
H, DH = 16, 64
P = 128
NCORES = 8
S_LOC = S // 4          # tokens per core: 512
KC = NV // P            # contraction chunks: 8
MC = S_LOC // P         # token chunks per core: 4
FC = NV // P            # output-feature chunks: 8
SCALE = 1.0 / 8.0       # 1 / sqrt(DH)

REPLICA_GROUPS = [[0, 1, 2, 3], [4, 5, 6, 7]]

# float32r (= TF32) runs the PE 4x faster than fp32 (1 cycle/row at N>=256).
# The BIR verifier requires f32r matmul inputs to be *produced* as f32r, so
# f32r operand tensors are declared float32r end-to-end and the host
# pre-rounds their values to TF32 (the DMA is then dtype-preserving).
#
# The output projection always runs in TF32: X is post-softmax, so its
# rounding is not amplified (measured 2.4e-4 of output scale).  The
# pre-softmax chain (Q/K/V projections and K^T V) feeds the softmax
# exponent, where TF32 rounding amplifies to ~1e-2 of output scale;
# PRE_TF32 selects fp32 (safe) or TF32 (fast) for that chain.
PRE_TF32 = os.environ.get("KERNEL_PRE_TF32", "1") == "1"
PRE_DT = F32R if PRE_TF32 else F32


def _round_dt(a, dt_):
    if dt_ != F32R:
        return np.ascontiguousarray(a, dtype=np.float32)
    b = np.ascontiguousarray(a, dtype=np.float32).view(np.uint32)
    lsb = (b >> np.uint32(13)) & np.uint32(1)
    out = (b + np.uint32(0x0FFF) + lsb) & np.uint32(0xFFFFE000)
    return out.view(np.float32)


def build_nc(collective=True):
    nc = bacc.Bacc("TRN2", target_bir_lowering=False, debug=False,
                   num_devices=NCORES if collective else 1)

    sqT = nc.dram_tensor("sqT", [NV, S_LOC], PRE_DT, kind="ExternalInput").ap()
    skT = nc.dram_tensor("skT", [NV, S_LOC], PRE_DT, kind="ExternalInput").ap()
    svT = nc.dram_tensor("svT", [NV, S_LOC], PRE_DT, kind="ExternalInput").ap()
    wqT = nc.dram_tensor("wqT", [NV, NV], PRE_DT, kind="ExternalInput").ap()
    owT = nc.dram_tensor("owT", [NV, NV], F32R, kind="ExternalInput").ap()
    bq = nc.dram_tensor("bq", [NV], F32, kind="ExternalInput").ap()
    ob = nc.dram_tensor("ob", [NV], F32, kind="ExternalInput").ap()
    zT = nc.dram_tensor("zT", [NV, S_LOC], F32, kind="ExternalOutput").ap()

    with tile.TileContext(nc) as tc:
        _emit(nc, tc, sqT, skT, svT, wqT, owT, bq, ob, zT,
              collective=collective)

    nc.compile()
    return nc


def _emit(nc, tc, sqT, skT, svT, wqT, owT, bq, ob, zT, collective=True):
    from contextlib import ExitStack

    with ExitStack() as ctx:
        persist = ctx.enter_context(tc.tile_pool(name="persist", bufs=1))
        recyc = ctx.enter_context(tc.tile_pool(name="recyc", bufs=1))
        scratch = ctx.enter_context(tc.tile_pool(name="scratch", bufs=2))
        psproj = ctx.enter_context(
            tc.tile_pool(name="psproj", bufs=2, space="PSUM"))
        dram = ctx.enter_context(tc.tile_pool(name="dram", bufs=1, space="DRAM"))

        # ---- constant / weight loads ------------------------------------
        # DMA issue order is the critical path at kernel start: the first
        # quarter of Wq plus Sk (phase 1's first inputs, split so the PE can
        # start accumulating early), everything else behind, out_w last
        # (not needed until the final phase).
        wq_sb = persist.tile([P, KC, NV], PRE_DT)   # wq_sb[p,kc,n] = Wq_w[n, kc*128+p]
        wq_view = wqT.rearrange("(kc p) n -> p kc n", p=P)
        nc.sync.dma_start(wq_sb[:, :, 0:256], wq_view[:, :, 0:256])
        sk_sb = recyc.tile([P, KC, S_LOC], PRE_DT, tag="s1")
        sk_view = skT.rearrange("(kc p) m -> p kc m", p=P)
        nc.sync.dma_start(sk_sb[:, 0:KC // 2], sk_view[:, 0:KC // 2])
        nc.sync.dma_start(sk_sb[:, KC // 2:], sk_view[:, KC // 2:])
        nc.sync.dma_start(wq_sb[:, :, 256:512], wq_view[:, :, 256:512])
        nc.sync.dma_start(wq_sb[:, :, 512:NV], wq_view[:, :, 512:NV])

        bqp_sb = persist.tile([P, FC], F32)      # per-partition view for QT bias
        nc.sync.dma_start(bqp_sb[:], bq.rearrange("(c p) -> p c", p=P))
        obp_sb = persist.tile([P, FC], F32)
        nc.sync.dma_start(obp_sb[:], ob.rearrange("(c p) -> p c", p=P))
        bqr_sb = persist.tile([1, NV], F32)      # bias as a single row
        nc.sync.dma_start(bqr_sb[:], bq[None, :])
        # bias replicated across partitions, for the free-dim bias add on K/V
        bqb_sb = persist.tile([P, NV], F32)
        nc.gpsimd.partition_broadcast(bqb_sb[:], bqr_sb[:])

        ident_sb = persist.tile([P, P], F32)
        make_identity(nc, ident_sb[:])

        sv_sb = recyc.tile([P, KC, S_LOC], PRE_DT, tag="s2")
        sv_view = svT.rearrange("(kc p) m -> p kc m", p=P)
        nc.sync.dma_start(sv_sb[:, 0:KC // 2], sv_view[:, 0:KC // 2])
        nc.sync.dma_start(sv_sb[:, KC // 2:], sv_view[:, KC // 2:])
        sq_sb = recyc.tile([P, KC, S_LOC], PRE_DT, tag="s0")
        nc.sync.dma_start(sq_sb[:], sqT.rearrange("(kc p) m -> p kc m", p=P))
        ow_sb = persist.tile([P, KC, NV], F32R)
        nc.sync.dma_start(ow_sb[:], owT.rearrange("(kc p) n -> p kc n", p=P))

        q_sb = persist.tile([P, FC, S_LOC], F32)  # Q^T: feature on partition
        k_sb = persist.tile([P, MC, NV], PRE_DT)     # K natural: token on partition
        v_sb = persist.tile([P, MC, NV], PRE_DT)
        m_sb = persist.tile([P, NV], F32)         # local K^T V / 8, block-diag
        mr_sb = persist.tile([P, NV], F32)        # after AllReduce

        # ---- K, V projections in natural layout (token on partition) ----
        #   K[m, n] = sum_k Sk[m, k] Wq[n, k] + bq[n]
        #   lhsT = SkT chunk (k on part, token free) ; rhs = WqT chunk
        #   bias (varies along the free dim) folds into the PSUM evacuation
        for src, dst in ((sk_sb, k_sb), (sv_sb, v_sb)):
            for mc in range(MC):
                for half in range(2):
                    nsl = slice(half * 512, (half + 1) * 512)
                    ps = psproj.tile([P, 512], F32, tag="proj")
                    # the first half in quarter-wide groups: its weight
                    # quarters arrive first, so the PE starts sooner
                    nq = 2 if (src is sk_sb and half == 0 and mc == 0) else 1
                    for q in range(nq):
                        w = 512 // nq
                        psl = slice(q * w, (q + 1) * w)
                        qsl = slice(half * 512 + q * w, half * 512 + (q + 1) * w)
                        for kc in range(KC):
                            nc.tensor.matmul(
                                ps[:, psl],
                                (src[:, kc, mc * P:(mc + 1) * P]),
                                (wq_sb[:, kc, qsl]),
                                start=(kc == 0), stop=(kc == KC - 1))
                    nc.vector.tensor_tensor(
                        dst[:, mc, nsl], ps[:], bqb_sb[:, nsl], ALU.add)

        # ---- local M = K^T V / 8 per feature chunk ----------------------
        # feature chunk fc holds heads 2fc, 2fc+1 in its 64x64 diagonal
        # blocks.  N=256 keeps float32r at full rate; only the diagonal
        # blocks are copied out, the rest of m_sb stays zero.
        nc.vector.memset(m_sb[:], 0.0)
        with tc.tile_pool(name="psm", bufs=2, space="PSUM") as psm:
            for fc in range(FC):
                base = fc * P if fc % 2 == 0 else (fc - 1) * P
                c0 = fc * P - base
                ps = psm.tile([P, 256], F32, tag="m")
                for mc in range(MC):
                    nc.tensor.matmul(
                        ps[:],
                        (k_sb[:, mc, fc * P:(fc + 1) * P]),
                        (v_sb[:, mc, base:base + 256]),
                        start=(mc == 0), stop=(mc == MC - 1))
                # DVE, not ACT: the ACT queue is busy with Q-projection
                # bias evacuations, and M gates the collective
                nc.vector.tensor_scalar_mul(
                    m_sb[0:64, fc * P:fc * P + 64],
                    ps[0:64, c0:c0 + 64], SCALE)
                nc.vector.tensor_scalar_mul(
                    m_sb[64:128, fc * P + 64:fc * P + 128],
                    ps[64:128, c0 + 64:c0 + 128], SCALE)

        # ---- AllReduce M within each batch group (512 KB) ---------------
        m_in = dram.tile([P, NV], F32)
        m_out = dram.tile([P, NV], F32)  # Shared outputs need >4-core groups
        nc.sync.dma_start(m_in[:], m_sb[:])
        if collective:
            nc.gpsimd.collective_compute(
                "AllReduce", ALU.add,
                replica_groups=REPLICA_GROUPS,
                ins=[m_in.opt()], outs=[m_out.opt()])
        else:  # single-core perf-model variant: same traffic, no collective
            nc.sync.dma_start(m_out[:], m_in[:])
        nc.sync.dma_start(mr_sb[:], m_out[:])

        # ---- Q projection, transposed layout (feature on partition) -----
        # overlaps with the collective above (no data dependence)
        for fc in range(FC):
            ps = psproj.tile([P, 512], F32, tag="proj")
            for kc in range(KC):
                nc.tensor.matmul(
                    ps[:],
                    (wq_sb[:, kc, fc * P:(fc + 1) * P]),
                    (sq_sb[:, kc, :]),
                    start=(kc == 0), stop=(kc == KC - 1))
            nc.scalar.add(q_sb[:, fc, :], ps[:], bqp_sb[:, fc:fc + 1])

        # ---- Y = Q M (block-diag), segmented softmax, transpose, out-proj
        # interleaved per token chunk: PE transposes chunk mc while the DVE
        # runs chunk mc+1's softmax; the output projection runs per token
        # half so it starts before the last softmax finishes.
        x_sb = recyc.tile([P, MC, NV], F32, tag="s1")   # reuses sk_sb slot
        xT_sb = recyc.tile([P, KC, S_LOC], F32R, tag="s2")  # reuses sv_sb slot
        zT_sb = recyc.tile([P, KC, S_LOC], F32, tag="s0")  # reuses sq_sb slot
        zv = zT.rearrange("(fc p) m -> p fc m", p=P)
        with tc.tile_pool(name="psy", bufs=1, space="PSUM") as psy:
            for mc in range(MC):
                yps = psy.tile([P, NV], F32, tag="y", bufs=2)
                for fc in range(FC):
                    nc.tensor.matmul(
                        yps[:, fc * P:(fc + 1) * P],
                        q_sb[:, fc, mc * P:(mc + 1) * P],
                        mr_sb[:, fc * P:(fc + 1) * P],
                        start=True, stop=True)
                y3 = yps.rearrange("p (h d) -> p h d", d=DH)
                nmx = scratch.tile([P, H], F32, tag="nmx")
                nc.vector.reduce_max(nmx[:], y3, axis=mybir.AxisListType.X,
                                     negate=True)
                e_sb = scratch.tile([P, NV], F32, tag="e")
                e3 = e_sb.rearrange("p (h d) -> p h d", d=DH)
                nc.vector.tensor_tensor(
                    e3, y3, nmx[:, :, None].to_broadcast((P, H, DH)), ALU.add)
                nc.scalar.activation(e_sb[:], e_sb[:], AF.Exp)
                sm = scratch.tile([P, H], F32, tag="sm")
                nc.vector.reduce_sum(sm[:], e3, axis=mybir.AxisListType.X)
                rc = scratch.tile([P, H], F32, tag="rc")
                nc.vector.reciprocal(rc[:], sm[:])
                x3 = x_sb[:, mc, :].rearrange("p (h d) -> p h d", d=DH)
                HH = H // 2
                if mc < 2:
                    nc.vector.tensor_tensor(
                        x3[:, 0:HH], e3[:, 0:HH],
                        rc[:, 0:HH, None].to_broadcast((P, HH, DH)), ALU.mult)
                    nc.gpsimd.tensor_tensor(
                        x3[:, HH:H], e3[:, HH:H],
                        rc[:, HH:H, None].to_broadcast((P, HH, DH)), ALU.mult)
                else:
                    # late chunks: the DVE chain tail gates the transposes,
                    # so normalize fully on the idle GPSIMD
                    nc.gpsimd.tensor_tensor(
                        x3, e3, rc[:, :, None].to_broadcast((P, H, DH)),
                        ALU.mult)

                # transpose this token chunk back to feature-on-partition
                for th in range(2):
                    pst = psy.tile([P, 512], F32, tag="tp", bufs=2)
                    for f in range(4):
                        fc = th * 4 + f
                        nc.tensor.transpose(
                            pst[:, f * P:(f + 1) * P],
                            x_sb[:, mc, fc * P:(fc + 1) * P], ident_sb[:])
                    nc.scalar.copy(
                        xT_sb[:, th * 4:(th + 1) * 4, mc * P:(mc + 1) * P],
                        pst.rearrange("p (fc m) -> p fc m", m=P))

            # ---- output projection: token halves overlap the late softmax
            for half in range(2):
                msl = slice(half * 256, (half + 1) * 256)
                for fc in range(FC):
                    ps = psproj.tile([P, 512], F32, tag="proj")
                    for kc in range(KC):
                        nc.tensor.matmul(
                            ps[:, 0:256],
                            (ow_sb[:, kc, fc * P:(fc + 1) * P]),
                            (xT_sb[:, kc, msl]),
                            start=(kc == 0), stop=(kc == KC - 1))
                    nc.scalar.add(zT_sb[:, fc, msl], ps[:, 0:256],
                                  obp_sb[:, fc:fc + 1])
                    nc.sync.dma_start(zv[:, fc, msl], zT_sb[:, fc, msl])

_NC_CACHE = None


def _get_nc():
    global _NC_CACHE
    if _NC_CACHE is None:
        _NC_CACHE = build_nc()
    return _NC_CACHE


def make_in_maps(Sq, Sk, Sv, Wq_w, Wq_b, out_w, out_b):
    wqT = _round_dt(np.asarray(Wq_w, dtype=np.float32).T, PRE_DT)
    owT = _round_dt(np.asarray(out_w, dtype=np.float32).T, F32R)
    bq = np.asarray(Wq_b, dtype=np.float32)
    ob = np.asarray(out_b, dtype=np.float32)
    in_maps = []
    for c in range(NCORES):
        b, q = c // 4, c % 4
        rows = slice(q * S_LOC, (q + 1) * S_LOC)
        in_maps.append({
            "sqT": _round_dt(np.asarray(Sq[b, rows], np.float32).T, PRE_DT),
            "skT": _round_dt(np.asarray(Sk[b, rows], np.float32).T, PRE_DT),
            "svT": _round_dt(np.asarray(Sv[b, rows], np.float32).T, PRE_DT),
            "wqT": wqT, "owT": owT, "bq": bq, "ob": ob,
        })
    return in_maps


def gather_output(results):
    out = np.empty((B, S, NV), dtype=np.float32)
    for c in range(NCORES):
        b, q = c // 4, c % 4
        out[b, q * S_LOC:(q + 1) * S_LOC, :] = results[c]["zT"].T
    return out


def kernel(Sq, Sk, Sv, Wq_w, Wq_b, out_w, out_b, **_unused):
    nc = _get_nc()
    in_maps = make_in_maps(Sq, Sk, Sv, Wq_w, Wq_b, out_w, out_b)
    res = run_bass_kernel_spmd(nc, in_maps, core_ids=list(range(NCORES)))
    return gather_output(res.results)



# revision 4
# speedup vs baseline: 1.1092x; 1.1092x over previous
"""Trainium2 Bass kernel for nn_MultiHeadAttention_20066087207431.

Reference computation (B=2, S=2048, NV=1024, H=16, DH=64):
    Q = Sq @ Wq_w.T + Wq_b ; K = Sk @ Wq_w.T + Wq_b ; V = Sv @ Wq_w.T + Wq_b
    per (batch, head):  qk = Q K^T / sqrt(DH) ;  Y = qk @ V
    X = softmax(Y, axis=-1)          # softmax AFTER the second matmul (quirk)
    out = X @ out_w.T + out_b

Key algebraic optimization: there is no softmax between the two attention
matmuls, so (Q K^T) V == Q (K^T V).  K^T V is only (64, 64) per head, which
collapses ~34 GFLOP of score math into ~1 GFLOP and removes the (S, S) score
matrices entirely.

Sharding: 8 cores; core c handles batch b = c // 4 and a 512-token slice of S
(data-parallel over batch*sequence).  Each core computes local partial
M = K_loc^T V_loc / sqrt(DH) for all 16 heads; a 256 KB AllReduce within each
4-core batch group completes the sum over S.  Everything else (projections,
Y = Q M, softmax, output projection) is local to the core.  Weights are
replicated.

Dtype: fp16 everywhere on the PE.  fp16 has the same 10-bit mantissa as TF32
(f32r), so accuracy matches the TF32 baseline, while:
  - HBM traffic halves (inputs/weights/output are 2 bytes),
  - matmuls run at 1 cycle/row with NO N>=256 restriction (f32r needs
    N>=256 else 4x penalty), so the N=128 attention matmuls are full rate,
  - PE transposes run at 1.0 cycles/row instead of 1.5.
PSUM accumulation stays fp32; biases are added in fp32 during PSUM
evacuation, then rounded once to fp16.

On-device layouts (SBUF is 128 partitions x free):
    K/V are produced in natural layout (token on partition) so the M matmuls
    can contract over tokens; Q is kept transposed (feature on partition).
    The final output is computed in natural layout (token on partition) so
    the host gathers without a transpose.  M is stored block-diagonal
    ((128,1024), head pair per 128-col chunk, zeros off-diagonal).
"""

import os
import sys

import numpy as np

for _p in ("/opt/trn_rl_repo", "/root/.axon_site/_ro/trn_rl_repo"):
    if os.path.isdir(_p) and _p not in sys.path:
        sys.path.insert(0, _p)

import concourse.bass as bass  # noqa: E402
import concourse.mybir as mybir  # noqa: E402
import concourse.tile as tile  # noqa: E402
from concourse import bacc  # noqa: E402
from concourse.bass_utils import run_bass_kernel_spmd  # noqa: E402
from concourse.masks import make_identity  # noqa: E402

F32 = mybir.dt.float32
F16 = mybir.dt.float16
AF = mybir.ActivationFunctionType
ALU = mybir.AluOpType

B, S, NV = 2, 2048, 1024
H, DH = 16, 64
P = 128
NCORES = 8
S_LOC = S // 4          # tokens per core: 512
KC = NV // P            # contraction chunks: 8
MC = S_LOC // P         # token chunks per core: 4
FC = NV // P            # output-feature chunks: 8
SCALE = 1.0 / 8.0       # 1 / sqrt(DH)

REPLICA_GROUPS = [[0, 1, 2, 3], [4, 5, 6, 7]]


def build_nc(collective=True):
    nc = bacc.Bacc("TRN2", target_bir_lowering=False, debug=False,
                   num_devices=NCORES if collective else 1)

    sqT = nc.dram_tensor("sqT", [NV, S_LOC], F16, kind="ExternalInput").ap()
    skT = nc.dram_tensor("skT", [NV, S_LOC], F16, kind="ExternalInput").ap()
    svT = nc.dram_tensor("svT", [NV, S_LOC], F16, kind="ExternalInput").ap()
    wqT = nc.dram_tensor("wqT", [NV, NV], F16, kind="ExternalInput").ap()
    owT = nc.dram_tensor("owT", [NV, NV], F16, kind="ExternalInput").ap()
    bq = nc.dram_tensor("bq", [NV], F32, kind="ExternalInput").ap()
    ob = nc.dram_tensor("ob", [NV], F32, kind="ExternalInput").ap()
    z = nc.dram_tensor("z", [S_LOC, NV], F16, kind="ExternalOutput").ap()

    with tile.TileContext(nc) as tc:
        _emit(nc, tc, sqT, skT, svT, wqT, owT, bq, ob, z,
              collective=collective)

    nc.compile()
    return nc


def _emit(nc, tc, sqT, skT, svT, wqT, owT, bq, ob, z, collective=True):
    from contextlib import ExitStack

    with ExitStack() as ctx:
        persist = ctx.enter_context(tc.tile_pool(name="persist", bufs=1))
        scratch = ctx.enter_context(tc.tile_pool(name="scratch", bufs=2))
        psproj = ctx.enter_context(
            tc.tile_pool(name="psproj", bufs=2, space="PSUM"))
        dram = ctx.enter_context(tc.tile_pool(name="dram", bufs=1, space="DRAM"))

        # ---- constant / weight loads ------------------------------------
        # DMA issue order is the critical path at kernel start: the first
        # quarter of Wq plus Sk (phase 1's first inputs) first, everything
        # else behind, out_w last (not needed until the final phase).
        wq_sb = persist.tile([P, KC, NV], F16)   # wq_sb[p,kc,n] = Wq_w[n, kc*128+p]
        wq_view = wqT.rearrange("(kc p) n -> p kc n", p=P)
        nc.sync.dma_start(wq_sb[:, :, 0:256], wq_view[:, :, 0:256])
        sk_sb = persist.tile([P, KC, S_LOC], F16)
        sk_view = skT.rearrange("(kc p) m -> p kc m", p=P)
        nc.sync.dma_start(sk_sb[:, 0:KC // 2], sk_view[:, 0:KC // 2])
        nc.sync.dma_start(sk_sb[:, KC // 2:], sk_view[:, KC // 2:])
        nc.sync.dma_start(wq_sb[:, :, 256:512], wq_view[:, :, 256:512])

        bqp_sb = persist.tile([P, FC], F32)      # per-partition view for QT bias
        nc.sync.dma_start(bqp_sb[:], bq.rearrange("(c p) -> p c", p=P))
        bqr_sb = persist.tile([1, NV], F32)      # bias as a single row
        nc.sync.dma_start(bqr_sb[:], bq[None, :])
        obr_sb = persist.tile([1, NV], F32)
        nc.sync.dma_start(obr_sb[:], ob[None, :])
        # bias replicated across partitions, for the free-dim bias add on K/V/z
        bqb_sb = persist.tile([P, NV], F32)
        nc.gpsimd.partition_broadcast(bqb_sb[:], bqr_sb[:])
        obb_sb = persist.tile([P, NV], F32)
        nc.gpsimd.partition_broadcast(obb_sb[:], obr_sb[:])

        ident_sb = persist.tile([P, P], F16)
        make_identity(nc, ident_sb[:])

        nc.sync.dma_start(wq_sb[:, :, 512:NV], wq_view[:, :, 512:NV])
        sv_sb = persist.tile([P, KC, S_LOC], F16)
        sv_view = svT.rearrange("(kc p) m -> p kc m", p=P)
        nc.sync.dma_start(sv_sb[:, 0:KC // 2], sv_view[:, 0:KC // 2])
        nc.sync.dma_start(sv_sb[:, KC // 2:], sv_view[:, KC // 2:])
        sq_sb = persist.tile([P, KC, S_LOC], F16)
        nc.sync.dma_start(sq_sb[:], sqT.rearrange("(kc p) m -> p kc m", p=P))
        ow_sb = persist.tile([P, KC, NV], F16)
        ow_view = owT.rearrange("(kc p) n -> p kc n", p=P)
        nc.sync.dma_start(ow_sb[:, 0:KC // 2], ow_view[:, 0:KC // 2])
        nc.sync.dma_start(ow_sb[:, KC // 2:], ow_view[:, KC // 2:])

        q_sb = persist.tile([P, FC, S_LOC], F16)  # Q^T: feature on partition
        k_sb = persist.tile([P, MC, NV], F16)     # K natural: token on partition
        v_sb = persist.tile([P, MC, NV], F16)
        m_sb = persist.tile([P, NV], F16)         # local K^T V / 8, block-diag
        mr_sb = persist.tile([P, NV], F16)        # after AllReduce

        # ---- K, V projections in natural layout (token on partition) ----
        #   K[m, n] = sum_k Sk[m, k] Wq[n, k] + bq[n]
        #   lhsT = SkT chunk (k on part, token free) ; rhs = WqT chunk
        #   bias (varies along the free dim) folds into the PSUM evacuation
        # both evacs on DVE: GPSIMD cannot read PSUM, ACT cannot add a
        # free-dim-varying bias
        for src, dst, evac in ((sk_sb, k_sb, "dve"), (sv_sb, v_sb, "dve")):
            for mc in range(MC):
                for half in range(2):
                    nsl = slice(half * 512, (half + 1) * 512)
                    ps = psproj.tile([P, 512], F32, tag="proj")
                    # the first output: its weight quarters arrive first, so
                    # the PE starts sooner
                    nq = 2 if (src is sk_sb and half == 0 and mc == 0) else 1
                    for q in range(nq):
                        w = 512 // nq
                        psl = slice(q * w, (q + 1) * w)
                        qsl = slice(half * 512 + q * w, half * 512 + (q + 1) * w)
                        for kc in range(KC):
                            nc.tensor.matmul(
                                ps[:, psl],
                                (src[:, kc, mc * P:(mc + 1) * P]),
                                (wq_sb[:, kc, qsl]),
                                start=(kc == 0), stop=(kc == KC - 1))
                    if evac == "dve":
                        nc.vector.tensor_tensor(
                            dst[:, mc, nsl], ps[:], bqb_sb[:, nsl], ALU.add)
                    else:
                        nc.gpsimd.tensor_tensor(
                            dst[:, mc, nsl], ps[:], bqb_sb[:, nsl], ALU.add)

        # ---- local M = K^T V / 8 per feature chunk ----------------------
        # feature chunk fc holds heads 2fc, 2fc+1 in its 64x64 diagonal
        # blocks.  fp16 runs full rate at N=128, so compute only the fc x fc
        # 128x128 product (both diagonal 64x64 blocks live inside it).
        nc.vector.memset(m_sb[:], 0.0)
        with tc.tile_pool(name="psm", bufs=2, space="PSUM") as psm:
            for fc in range(FC):
                ps = psm.tile([P, P], F32, tag="m")
                for mc in range(MC):
                    nc.tensor.matmul(
                        ps[:],
                        (k_sb[:, mc, fc * P:(fc + 1) * P]),
                        (v_sb[:, mc, fc * P:(fc + 1) * P]),
                        start=(mc == 0), stop=(mc == MC - 1))
                # DVE, not ACT: the ACT queue is busy with Q-projection
                # bias evacuations, and M gates the collective
                nc.vector.tensor_scalar_mul(
                    m_sb[0:64, fc * P:fc * P + 64],
                    ps[0:64, 0:64], SCALE)
                nc.vector.tensor_scalar_mul(
                    m_sb[64:128, fc * P + 64:fc * P + 128],
                    ps[64:128, 64:128], SCALE)

        # ---- AllReduce M within each batch group (256 KB fp16) ----------
        m_in = dram.tile([P, NV], F16)
        m_out = dram.tile([P, NV], F16)
        nc.sync.dma_start(m_in[:], m_sb[:])
        if collective:
            nc.gpsimd.collective_compute(
                "AllReduce", ALU.add,
                replica_groups=REPLICA_GROUPS,
                ins=[m_in.opt()], outs=[m_out.opt()])
        else:  # single-core perf-model variant: same traffic, no collective
            nc.sync.dma_start(m_out[:], m_in[:])
        nc.sync.dma_start(mr_sb[:], m_out[:])

        # ---- Q projection, transposed layout (feature on partition) -----
        # overlaps with the collective above (no data dependence)
        for fc in range(FC):
            ps = psproj.tile([P, 512], F32, tag="proj")
            for kc in range(KC):
                nc.tensor.matmul(
                    ps[:],
                    (wq_sb[:, kc, fc * P:(fc + 1) * P]),
                    (sq_sb[:, kc, :]),
                    start=(kc == 0), stop=(kc == KC - 1))
            nc.scalar.add(q_sb[:, fc, :], ps[:], bqp_sb[:, fc:fc + 1])

        # ---- Y = Q M (block-diag), segmented softmax, transpose, out-proj
        # interleaved per token chunk: PE transposes chunk mc while the DVE
        # runs chunk mc+1's softmax; the output projection runs per token
        # chunk so it starts before the last softmax finishes.
        x_sb = persist.tile([P, MC, NV], F16)       # softmax output, natural
        xT_sb = persist.tile([P, KC, S_LOC], F16)   # X^T: feature on partition
        z_sb = persist.tile([P, MC, NV], F16)       # output, natural layout
        zv = z.rearrange("(mc p) n -> p mc n", p=P)
        with tc.tile_pool(name="psy", bufs=1, space="PSUM") as psy:
            for mc in range(MC):
                yps = psy.tile([P, NV], F32, tag="y", bufs=2)
                for fc in range(FC):
                    nc.tensor.matmul(
                        yps[:, fc * P:(fc + 1) * P],
                        q_sb[:, fc, mc * P:(mc + 1) * P],
                        mr_sb[:, fc * P:(fc + 1) * P],
                        start=True, stop=True)
                y3 = yps.rearrange("p (h d) -> p h d", d=DH)
                nmx = scratch.tile([P, H], F32, tag="nmx")
                nc.vector.reduce_max(nmx[:], y3, axis=mybir.AxisListType.X,
                                     negate=True)
                e_sb = scratch.tile([P, NV], F16, tag="e")
                e3 = e_sb.rearrange("p (h d) -> p h d", d=DH)
                nc.vector.tensor_tensor(
                    e3, y3, nmx[:, :, None].to_broadcast((P, H, DH)), ALU.add)
                nc.scalar.activation(e_sb[:], e_sb[:], AF.Exp)
                sm = scratch.tile([P, H], F32, tag="sm")
                nc.vector.reduce_sum(sm[:], e3, axis=mybir.AxisListType.X)
                rc = scratch.tile([P, H], F32, tag="rc")
                nc.vector.reciprocal(rc[:], sm[:])
                x3 = x_sb[:, mc, :].rearrange("p (h d) -> p h d", d=DH)
                HH = H // 2
                if mc < 2:
                    nc.vector.tensor_tensor(
                        x3[:, 0:HH], e3[:, 0:HH],
                        rc[:, 0:HH, None].to_broadcast((P, HH, DH)), ALU.mult)
                    nc.gpsimd.tensor_tensor(
                        x3[:, HH:H], e3[:, HH:H],
                        rc[:, HH:H, None].to_broadcast((P, HH, DH)), ALU.mult)
                else:
                    # late chunks: the DVE chain tail gates the transposes,
                    # so normalize fully on the idle GPSIMD
                    nc.gpsimd.tensor_tensor(
                        x3, e3, rc[:, :, None].to_broadcast((P, H, DH)),
                        ALU.mult)

                # transpose this token chunk back to feature-on-partition
                for th in range(2):
                    pst = psy.tile([P, 512], F16, tag="tp", bufs=2)
                    for f in range(4):
                        fc = th * 4 + f
                        nc.tensor.transpose(
                            pst[:, f * P:(f + 1) * P],
                            x_sb[:, mc, fc * P:(fc + 1) * P], ident_sb[:])
                    nc.scalar.copy(
                        xT_sb[:, th * 4:(th + 1) * 4, mc * P:(mc + 1) * P],
                        pst.rearrange("p (fc m) -> p fc m", m=P))

                # ---- output projection for this token chunk, natural z --
                #   z[m, n] = sum_k X[m, k] out_w[n, k] + ob[n]
                #   lhsT = X^T chunk (feat on part, token free); rhs = owT
                for half in range(2):
                    nsl = slice(half * 512, (half + 1) * 512)
                    ps = psproj.tile([P, 512], F32, tag="proj")
                    for kc in range(KC):
                        nc.tensor.matmul(
                            ps[:],
                            (xT_sb[:, kc, mc * P:(mc + 1) * P]),
                            (ow_sb[:, kc, nsl]),
                            start=(kc == 0), stop=(kc == KC - 1))
                    nc.vector.tensor_tensor(
                        z_sb[:, mc, nsl], ps[:], obb_sb[:, nsl], ALU.add)
                    nc.sync.dma_start(zv[:, mc, nsl], z_sb[:, mc, nsl])


_NC_CACHE = None


def _get_nc():
    global _NC_CACHE
    if _NC_CACHE is None:
        _NC_CACHE = build_nc()
    return _NC_CACHE


def make_in_maps(Sq, Sk, Sv, Wq_w, Wq_b, out_w, out_b):
    wqT = np.ascontiguousarray(
        np.asarray(Wq_w, dtype=np.float32).T.astype(np.float16))
    owT = np.ascontiguousarray(
        np.asarray(out_w, dtype=np.float32).T.astype(np.float16))
    bq = np.asarray(Wq_b, dtype=np.float32)
    ob = np.asarray(out_b, dtype=np.float32)
    in_maps = []
    for c in range(NCORES):
        b, q = c // 4, c % 4
        rows = slice(q * S_LOC, (q + 1) * S_LOC)
        in_maps.append({
            "sqT": np.ascontiguousarray(
                np.asarray(Sq[b, rows], np.float32).T.astype(np.float16)),
            "skT": np.ascontiguousarray(
                np.asarray(Sk[b, rows], np.float32).T.astype(np.float16)),
            "svT": np.ascontiguousarray(
                np.asarray(Sv[b, rows], np.float32).T.astype(np.float16)),
            "wqT": wqT, "owT": owT, "bq": bq, "ob": ob,
        })
    return in_maps


def gather_output(results):
    out = np.empty((B, S, NV), dtype=np.float32)
    for c in range(NCORES):
        b, q = c // 4, c % 4
        out[b, q * S_LOC:(q + 1) * S_LOC, :] = results[c]["z"].astype(
            np.float32)
    return out


def kernel(Sq, Sk, Sv, Wq_w, Wq_b, out_w, out_b, **_unused):
    nc = _get_nc()
    in_maps = make_in_maps(Sq, Sk, Sv, Wq_w, Wq_b, out_w, out_b)
    res = run_bass_kernel_spmd(nc, in_maps, core_ids=list(range(NCORES)))
    return gather_output(res.results)


# revision 5
# speedup vs baseline: 1.1761x; 1.0603x over previous
"""Trainium2 Bass kernel for nn_MultiHeadAttention_20066087207431.

Reference computation (B=2, S=2048, NV=1024, H=16, DH=64):
    Q = Sq @ Wq_w.T + Wq_b ; K = Sk @ Wq_w.T + Wq_b ; V = Sv @ Wq_w.T + Wq_b
    per (batch, head):  qk = Q K^T / sqrt(DH) ;  Y = qk @ V
    X = softmax(Y, axis=-1)          # softmax AFTER the second matmul (quirk)
    out = X @ out_w.T + out_b

Key algebraic optimizations:
  - No softmax between the two attention matmuls, so (Q K^T) V == Q (K^T V).
    K^T V is only (64, 64) per head, which collapses ~34 GFLOP of score math
    into ~1 GFLOP and removes the (S, S) score matrices entirely.
  - Softmax rows sum to exactly 1 per head (16 over the 1024 columns), so
    the output bias folds into the output weights on the host:
    X @ (out_w + out_b/16 * ones).T == X @ out_w.T + out_b.  The final PSUM
    evacuation is then a plain copy on the ACT engine.

Sharding: 8 cores; core c handles batch b = c // 4 and a 512-token slice of S
(data-parallel over batch*sequence).  Each core computes local partial
M = K_loc^T V_loc / sqrt(DH) for all 16 heads; a 256 KB AllReduce within each
4-core batch group completes the sum over S.  Everything else (projections,
Y = Q M, softmax, output projection) is local to the core.  Weights are
replicated.

Dtype: fp16 everywhere on the PE.  fp16 has the same 10-bit mantissa as TF32
(f32r), so accuracy matches the TF32 baseline, while:
  - HBM traffic halves (inputs/weights/output are 2 bytes),
  - matmuls run at 1 cycle/row with NO N>=256 restriction (f32r needs
    N>=256 else 4x penalty), so the N=128 attention matmuls are full rate,
  - PE transposes run at 1.0 cycles/row instead of 1.5.
PSUM accumulation stays fp32; biases are added in fp32 during PSUM
evacuation, then rounded once to fp16.

Schedule: the tail is pipelined per 128-token chunk with the PE one chunk
ahead: Y(0) Y(1) [sm(0) on DVE/ACT] T(0) O(0) Y(2) [sm(1)] T(1) O(1) ...
so the PE never stalls on a softmax.  Softmax is emitted in head-halves to
shorten the serial DVE chain; one normalize-mult half runs on GPSIMD.
"""

import os
import sys

import numpy as np

for _p in ("/opt/trn_rl_repo", "/root/.axon_site/_ro/trn_rl_repo"):
    if os.path.isdir(_p) and _p not in sys.path:
        sys.path.insert(0, _p)

import concourse.bass as bass  # noqa: E402
import concourse.mybir as mybir  # noqa: E402
import concourse.tile as tile  # noqa: E402
from concourse import bacc  # noqa: E402
from concourse.bass_utils import run_bass_kernel_spmd  # noqa: E402
from concourse.masks import make_identity  # noqa: E402

F32 = mybir.dt.float32
F16 = mybir.dt.float16
AF = mybir.ActivationFunctionType
ALU = mybir.AluOpType

B, S, NV = 2, 2048, 1024
H, DH = 16, 64
P = 128
NCORES = 8
S_LOC = S // 4          # tokens per core: 512
KC = NV // P            # contraction chunks: 8
MC = S_LOC // P         # token chunks per core: 4
FC = NV // P            # output-feature chunks: 8
SCALE = 1.0 / 8.0       # 1 / sqrt(DH)

REPLICA_GROUPS = [[0, 1, 2, 3], [4, 5, 6, 7]]


def build_nc(collective=True):
    nc = bacc.Bacc("TRN2", target_bir_lowering=False, debug=False,
                   num_devices=NCORES if collective else 1)

    sqT = nc.dram_tensor("sqT", [NV, S_LOC], F16, kind="ExternalInput").ap()
    skT = nc.dram_tensor("skT", [NV, S_LOC], F16, kind="ExternalInput").ap()
    svT = nc.dram_tensor("svT", [NV, S_LOC], F16, kind="ExternalInput").ap()
    wqT = nc.dram_tensor("wqT", [NV, NV], F16, kind="ExternalInput").ap()
    owT = nc.dram_tensor("owT", [NV, NV], F16, kind="ExternalInput").ap()
    bq = nc.dram_tensor("bq", [NV], F32, kind="ExternalInput").ap()
    z = nc.dram_tensor("z", [S_LOC, NV], F16, kind="ExternalOutput").ap()

    with tile.TileContext(nc) as tc:
        _emit(nc, tc, sqT, skT, svT, wqT, owT, bq, z,
              collective=collective)

    nc.compile()
    return nc


def _emit(nc, tc, sqT, skT, svT, wqT, owT, bq, z, collective=True):
    from contextlib import ExitStack

    with ExitStack() as ctx:
        persist = ctx.enter_context(tc.tile_pool(name="persist", bufs=1))
        scratch = ctx.enter_context(tc.tile_pool(name="scratch", bufs=2))
        psproj = ctx.enter_context(
            tc.tile_pool(name="psproj", bufs=2, space="PSUM"))
        dram = ctx.enter_context(tc.tile_pool(name="dram", bufs=1, space="DRAM"))

        # ---- constant / weight loads ------------------------------------
        # DMA issue order is the critical path at kernel start.  The first
        # K-projection output (mc 0, half 0) is emitted as four N=128 column
        # groups, so the PE can start after just sk tokens 0:128 + the first
        # 128 columns of Wq have landed (~3.5 us).  Everything else queues
        # behind in consumption order; out_w last.
        sk_sb = persist.tile([P, KC, S_LOC], F16)
        sk_view = skT.rearrange("(kc p) m -> p kc m", p=P)
        nc.sync.dma_start(sk_sb[:, :, 0:P], sk_view[:, :, 0:P])
        wq_sb = persist.tile([P, KC, NV], F16)   # wq_sb[p,kc,n] = Wq_w[n, kc*128+p]
        wq_view = wqT.rearrange("(kc p) n -> p kc n", p=P)
        nc.sync.dma_start(wq_sb[:, :, 0:P], wq_view[:, :, 0:P])
        nc.sync.dma_start(wq_sb[:, :, P:256], wq_view[:, :, P:256])
        nc.sync.dma_start(sk_sb[:, :, P:S_LOC], sk_view[:, :, P:S_LOC])
        nc.sync.dma_start(wq_sb[:, :, 256:512], wq_view[:, :, 256:512])

        bqp_sb = persist.tile([P, FC], F32)      # per-partition view for QT bias
        nc.sync.dma_start(bqp_sb[:], bq.rearrange("(c p) -> p c", p=P))
        bqr_sb = persist.tile([1, NV], F32)      # bias as a single row
        nc.sync.dma_start(bqr_sb[:], bq[None, :])
        # bias replicated across partitions, for the free-dim bias add on K/V
        bqb_sb = persist.tile([P, NV], F32)
        nc.gpsimd.partition_broadcast(bqb_sb[:], bqr_sb[:])

        ident_sb = persist.tile([P, P], F16)
        make_identity(nc, ident_sb[:])

        nc.sync.dma_start(wq_sb[:, :, 512:NV], wq_view[:, :, 512:NV])
        sv_sb = persist.tile([P, KC, S_LOC], F16)
        sv_view = svT.rearrange("(kc p) m -> p kc m", p=P)
        nc.sync.dma_start(sv_sb[:, 0:KC // 2], sv_view[:, 0:KC // 2])
        nc.sync.dma_start(sv_sb[:, KC // 2:], sv_view[:, KC // 2:])
        sq_sb = persist.tile([P, KC, S_LOC], F16)
        nc.sync.dma_start(sq_sb[:], sqT.rearrange("(kc p) m -> p kc m", p=P))
        ow_sb = persist.tile([P, KC, NV], F16)   # out_w + ob/16 (host-folded)
        ow_view = owT.rearrange("(kc p) n -> p kc n", p=P)
        nc.sync.dma_start(ow_sb[:, 0:KC // 2], ow_view[:, 0:KC // 2])
        nc.sync.dma_start(ow_sb[:, KC // 2:], ow_view[:, KC // 2:])

        q_sb = persist.tile([P, FC, S_LOC], F16)  # Q^T: feature on partition
        k_sb = persist.tile([P, MC, NV], F16)     # K natural: token on partition
        v_sb = persist.tile([P, MC, NV], F16)
        m_sb = persist.tile([P, NV], F16)         # local K^T V / 8, block-diag
        mr_sb = persist.tile([P, NV], F16)        # after AllReduce

        # ---- K, V projections in natural layout (token on partition) ----
        #   K[m, n] = sum_k Sk[m, k] Wq[n, k] + bq[n]
        #   lhsT = SkT chunk (k on part, token free) ; rhs = WqT chunk
        #   bias (varies along the free dim) folds into the PSUM evacuation
        #   (DVE: GPSIMD cannot read PSUM, ACT cannot add a free-dim bias)
        for src, dst in ((sk_sb, k_sb), (sv_sb, v_sb)):
            for mc in range(MC):
                for half in range(2):
                    nsl = slice(half * 512, (half + 1) * 512)
                    ps = psproj.tile([P, 512], F32, tag="proj")
                    # first output in N=128 column groups: each needs only one
                    # eighth of Wq's columns, so the PE starts sooner
                    nq = 4 if (src is sk_sb and half == 0 and mc == 0) else 1
                    for q in range(nq):
                        w = 512 // nq
                        psl = slice(q * w, (q + 1) * w)
                        qsl = slice(half * 512 + q * w, half * 512 + (q + 1) * w)
                        for kc in range(KC):
                            nc.tensor.matmul(
                                ps[:, psl],
                                (src[:, kc, mc * P:(mc + 1) * P]),
                                (wq_sb[:, kc, qsl]),
                                start=(kc == 0), stop=(kc == KC - 1))
                    nc.vector.tensor_tensor(
                        dst[:, mc, nsl], ps[:], bqb_sb[:, nsl], ALU.add)

        # ---- local M = K^T V / 8 per feature chunk ----------------------
        # feature chunk fc holds heads 2fc, 2fc+1 in its 64x64 diagonal
        # blocks.  fp16 runs full rate at N=128, so compute only the fc x fc
        # 128x128 product (both diagonal 64x64 blocks live inside it).
        nc.vector.memset(m_sb[:], 0.0)
        with tc.tile_pool(name="psm", bufs=2, space="PSUM") as psm:
            for fc in range(FC):
                ps = psm.tile([P, P], F32, tag="m")
                for mc in range(MC):
                    nc.tensor.matmul(
                        ps[:],
                        (k_sb[:, mc, fc * P:(fc + 1) * P]),
                        (v_sb[:, mc, fc * P:(fc + 1) * P]),
                        start=(mc == 0), stop=(mc == MC - 1))
                # DVE, not ACT: the ACT queue is busy with Q-projection
                # bias evacuations, and M gates the collective
                nc.vector.tensor_scalar_mul(
                    m_sb[0:64, fc * P:fc * P + 64],
                    ps[0:64, 0:64], SCALE)
                nc.vector.tensor_scalar_mul(
                    m_sb[64:128, fc * P + 64:fc * P + 128],
                    ps[64:128, 64:128], SCALE)

        # ---- AllReduce M within each batch group (256 KB fp16) ----------
        m_in = dram.tile([P, NV], F16)
        m_out = dram.tile([P, NV], F16)
        nc.sync.dma_start(m_in[:], m_sb[:])
        if collective:
            nc.gpsimd.collective_compute(
                "AllReduce", ALU.add,
                replica_groups=REPLICA_GROUPS,
                ins=[m_in.opt()], outs=[m_out.opt()])
        else:  # single-core perf-model variant: same traffic, no collective
            nc.sync.dma_start(m_out[:], m_in[:])
        nc.sync.dma_start(mr_sb[:], m_out[:])

        # ---- Q projection, transposed layout (feature on partition) -----
        # overlaps with the collective above (no data dependence)
        for fc in range(FC):
            ps = psproj.tile([P, 512], F32, tag="proj")
            for kc in range(KC):
                nc.tensor.matmul(
                    ps[:],
                    (wq_sb[:, kc, fc * P:(fc + 1) * P]),
                    (sq_sb[:, kc, :]),
                    start=(kc == 0), stop=(kc == KC - 1))
            nc.scalar.add(q_sb[:, fc, :], ps[:], bqp_sb[:, fc:fc + 1])

        # ---- Y = Q M (block-diag), segmented softmax, transpose, out-proj
        # pipelined per token chunk with the PE one chunk ahead (see module
        # docstring).  Softmax is split into head-halves; half 1's
        # normalize-mult runs on the otherwise idle GPSIMD.
        x_sb = persist.tile([P, MC, NV], F16)       # softmax output, natural
        xT_sb = persist.tile([P, KC, S_LOC], F16)   # X^T: feature on partition
        z_sb = persist.tile([P, MC, NV], F16)       # output, natural layout
        zv = z.rearrange("(mc p) n -> p mc n", p=P)

        with tc.tile_pool(name="psy", bufs=1, space="PSUM") as psy:

            def emit_y(mc):
                yps = psy.tile([P, NV], F32, tag="y", bufs=2)
                for fc in range(FC):
                    nc.tensor.matmul(
                        yps[:, fc * P:(fc + 1) * P],
                        q_sb[:, fc, mc * P:(mc + 1) * P],
                        mr_sb[:, fc * P:(fc + 1) * P],
                        start=True, stop=True)
                return yps

            def emit_softmax(mc, yps):
                y3 = yps.rearrange("p (h d) -> p h d", d=DH)
                x3 = x_sb[:, mc, :].rearrange("p (h d) -> p h d", d=DH)
                HH = H // 2
                for hh in range(2):
                    hsl = slice(hh * HH, (hh + 1) * HH)
                    esl = slice(hh * 512, (hh + 1) * 512)
                    nmx = scratch.tile([P, HH], F32, tag=f"nmx{hh}")
                    nc.vector.reduce_max(nmx[:], y3[:, hsl],
                                         axis=mybir.AxisListType.X,
                                         negate=True)
                    e_sb = scratch.tile([P, 512], F16, tag=f"e{hh}")
                    e3 = e_sb.rearrange("p (h d) -> p h d", d=DH)
                    nc.vector.tensor_tensor(
                        e3, y3[:, hsl],
                        nmx[:, :, None].to_broadcast((P, HH, DH)), ALU.add)
                    nc.scalar.activation(e_sb[:], e_sb[:], AF.Exp)
                    sm = scratch.tile([P, HH], F32, tag=f"sm{hh}")
                    nc.vector.reduce_sum(sm[:], e3, axis=mybir.AxisListType.X)
                    rc = scratch.tile([P, HH], F32, tag=f"rc{hh}")
                    nc.vector.reciprocal(rc[:], sm[:])
                    rcb = rc[:, :, None].to_broadcast((P, HH, DH))
                    if hh == 0:
                        nc.vector.tensor_tensor(x3[:, hsl], e3, rcb, ALU.mult)
                    else:
                        nc.gpsimd.tensor_tensor(x3[:, hsl], e3, rcb, ALU.mult)

            def emit_to(mc):
                # transpose token chunk mc back to feature-on-partition, then
                # project.  th=0 covers heads 0-7 (gated by the DVE mult),
                # th=1 heads 8-15 (gated by the GPSIMD mult).
                for th in range(2):
                    pst = psy.tile([P, 512], F16, tag="tp", bufs=2)
                    for f in range(4):
                        fc = th * 4 + f
                        nc.tensor.transpose(
                            pst[:, f * P:(f + 1) * P],
                            x_sb[:, mc, fc * P:(fc + 1) * P], ident_sb[:])
                    nc.scalar.copy(
                        xT_sb[:, th * 4:(th + 1) * 4, mc * P:(mc + 1) * P],
                        pst.rearrange("p (fc m) -> p fc m", m=P))
                #   z[m, n] = sum_k X[m, k] (out_w[n, k] + ob[n]/16)
                #   lhsT = X^T chunk (feat on part, token free); rhs = owT
                for half in range(2):
                    nsl = slice(half * 512, (half + 1) * 512)
                    ps = psproj.tile([P, 512], F32, tag="proj")
                    for kc in range(KC):
                        nc.tensor.matmul(
                            ps[:],
                            (xT_sb[:, kc, mc * P:(mc + 1) * P]),
                            (ow_sb[:, kc, nsl]),
                            start=(kc == 0), stop=(kc == KC - 1))
                    nc.scalar.copy(z_sb[:, mc, nsl], ps[:])
                    nc.sync.dma_start(zv[:, mc, nsl], z_sb[:, mc, nsl])

            for mc in range(MC):
                yps = emit_y(mc)
                emit_softmax(mc, yps)
                if mc >= 1:
                    emit_to(mc - 1)
            emit_to(MC - 1)


_NC_CACHE = None


def _get_nc():
    global _NC_CACHE
    if _NC_CACHE is None:
        _NC_CACHE = build_nc()
    return _NC_CACHE


def make_in_maps(Sq, Sk, Sv, Wq_w, Wq_b, out_w, out_b):
    wqT = np.ascontiguousarray(
        np.asarray(Wq_w, dtype=np.float32).T.astype(np.float16))
    # output bias folded into the output weights: softmax rows sum to
    # exactly 1 per head and there are 16 heads, so X @ (ow + ob/16).T
    # == X @ ow.T + ob
    ow_f = np.asarray(out_w, dtype=np.float32) + \
        np.asarray(out_b, dtype=np.float32)[:, None] / np.float32(H)
    owT = np.ascontiguousarray(ow_f.T.astype(np.float16))
    bq = np.asarray(Wq_b, dtype=np.float32)
    in_maps = []
    for c in range(NCORES):
        b, q = c // 4, c % 4
        rows = slice(q * S_LOC, (q + 1) * S_LOC)
        in_maps.append({
            "sqT": np.ascontiguousarray(
                np.asarray(Sq[b, rows], np.float32).T.astype(np.float16)),
            "skT": np.ascontiguousarray(
                np.asarray(Sk[b, rows], np.float32).T.astype(np.float16)),
            "svT": np.ascontiguousarray(
                np.asarray(Sv[b, rows], np.float32).T.astype(np.float16)),
            "wqT": wqT, "owT": owT, "bq": bq,
        })
    return in_maps


def gather_output(results):
    out = np.empty((B, S, NV), dtype=np.float32)
    for c in range(NCORES):
        b, q = c // 4, c % 4
        out[b, q * S_LOC:(q + 1) * S_LOC, :] = results[c]["z"].astype(
            np.float32)
    return out


def kernel(Sq, Sk, Sv, Wq_w, Wq_b, out_w, out_b, **_unused):
    nc = _get_nc()
    in_maps = make_in_maps(Sq, Sk, Sv, Wq_w, Wq_b, out_w, out_b)
    res = run_bass_kernel_spmd(nc, in_maps, core_ids=list(range(NCORES)))
    return gather_output(res.results)


# revision 10
# speedup vs baseline: 1.2282x; 1.0443x over previous
"""Trainium2 Bass kernel for nn_MultiHeadAttention_20066087207431.

Reference computation (B=2, S=2048, NV=1024, H=16, DH=64):
    Q = Sq @ Wq_w.T + Wq_b ; K = Sk @ Wq_w.T + Wq_b ; V = Sv @ Wq_w.T + Wq_b
    per (batch, head):  qk = Q K^T / sqrt(DH) ;  Y = qk @ V
    X = softmax(Y, axis=-1)          # softmax AFTER the second matmul (quirk)
    out = X @ out_w.T + out_b

Key algebraic optimizations:
  - No softmax between the two attention matmuls, so (Q K^T) V == Q (K^T V).
    K^T V is only (64, 64) per head, which collapses ~34 GFLOP of score math
    into ~1 GFLOP and removes the (S, S) score matrices entirely.
  - Softmax rows sum to exactly 1 per head (16 over the 1024 columns), so
    the output bias folds into the output weights on the host:
    X @ (out_w + out_b/16 * ones).T == X @ out_w.T + out_b.  The final PSUM
    evacuation is then a plain copy on the ACT engine.

Sharding: 8 cores; core c handles batch b = c // 4 and a 512-token slice of S
(data-parallel over batch*sequence).  Each core computes local partial
M = K_loc^T V_loc / sqrt(DH) for all 16 heads; a 256 KB AllReduce within each
4-core batch group completes the sum over S.  Everything else (projections,
Y = Q M, softmax, output projection) is local to the core.  Weights are
replicated.

Dtype: fp16 everywhere on the PE.  fp16 has the same 10-bit mantissa as TF32
(f32r), so accuracy matches the TF32 baseline, while:
  - HBM traffic halves (inputs/weights/output are 2 bytes),
  - matmuls run at 1 cycle/row with NO N>=256 restriction (f32r needs
    N>=256 else 4x penalty), so the N=128 attention matmuls are full rate,
  - PE transposes run at 1.0 cycles/row instead of 1.5.
PSUM accumulation stays fp32; biases are added in fp32 during PSUM
evacuation, then rounded once to fp16.

Schedule: the tail is pipelined per 128-token chunk with the PE one chunk
ahead: Y(0) Y(1) [sm(0) on DVE/ACT] T(0) O(0) Y(2) [sm(1)] T(1) O(1) ...
so the PE never stalls on a softmax.  Softmax is emitted in head-halves to
shorten the serial DVE chain; one normalize-mult half runs on GPSIMD.
"""

import os
import sys

import numpy as np

for _p in ("/opt/trn_rl_repo", "/root/.axon_site/_ro/trn_rl_repo"):
    if os.path.isdir(_p) and _p not in sys.path:
        sys.path.insert(0, _p)

import concourse.bass as bass  # noqa: E402
import concourse.mybir as mybir  # noqa: E402
import concourse.tile as tile  # noqa: E402
from concourse import bacc  # noqa: E402
from concourse.bass_utils import run_bass_kernel_spmd  # noqa: E402
from concourse.masks import make_identity  # noqa: E402

F32 = mybir.dt.float32
F16 = mybir.dt.float16
AF = mybir.ActivationFunctionType
ALU = mybir.AluOpType

B, S, NV = 2, 2048, 1024
H, DH = 16, 64
P = 128
NCORES = 8
S_LOC = S // 4          # tokens per core: 512
KC = NV // P            # contraction chunks: 8
MC = S_LOC // P         # token chunks per core: 4
FC = NV // P            # output-feature chunks: 8
SCALE = 1.0 / 8.0       # 1 / sqrt(DH)

REPLICA_GROUPS = [[0, 1, 2, 3], [4, 5, 6, 7]]


def build_nc(collective=True):
    nc = bacc.Bacc("TRN2", target_bir_lowering=False, debug=False,
                   num_devices=NCORES if collective else 1)

    sqT = nc.dram_tensor("sqT", [NV, S_LOC], F16, kind="ExternalInput").ap()
    skT = nc.dram_tensor("skT", [NV, S_LOC], F16, kind="ExternalInput").ap()
    svT = nc.dram_tensor("svT", [NV, S_LOC], F16, kind="ExternalInput").ap()
    wqT = nc.dram_tensor("wqT", [NV, NV], F16, kind="ExternalInput").ap()
    owT = nc.dram_tensor("owT", [NV, NV], F16, kind="ExternalInput").ap()
    bq = nc.dram_tensor("bq", [NV], F32, kind="ExternalInput").ap()
    z = nc.dram_tensor("z", [S_LOC, NV], F16, kind="ExternalOutput").ap()

    with tile.TileContext(nc) as tc:
        _emit(nc, tc, sqT, skT, svT, wqT, owT, bq, z,
              collective=collective)

    nc.compile()
    return nc


def _emit(nc, tc, sqT, skT, svT, wqT, owT, bq, z, collective=True):
    from contextlib import ExitStack

    with ExitStack() as ctx:
        persist = ctx.enter_context(tc.tile_pool(name="persist", bufs=1))
        scratch = ctx.enter_context(tc.tile_pool(name="scratch", bufs=2))
        psproj = ctx.enter_context(
            tc.tile_pool(name="psproj", bufs=2, space="PSUM"))
        dram = ctx.enter_context(tc.tile_pool(name="dram", bufs=1, space="DRAM"))

        # ---- constant / weight loads ------------------------------------
        # DMA issue order is the critical path at kernel start.  Chunks keep
        # >=512-byte DRAM lines (256-byte-line DMAs run at half bandwidth).
        # The first K-projection output is emitted as two N=256 column
        # groups so the PE can start after sk half 0 + Wq cols 0:256
        # (~4.9 us).  Everything else queues in consumption order; out_w
        # last.
        sk_sb = persist.tile([P, KC, S_LOC], F16)
        sk_view = skT.rearrange("(kc p) m -> p kc m", p=P)
        nc.sync.dma_start(sk_sb[:, 0:KC // 2], sk_view[:, 0:KC // 2])
        wq_sb = persist.tile([P, KC, NV], F16)   # wq_sb[p,kc,n] = Wq_w[n, kc*128+p]
        wq_view = wqT.rearrange("(kc p) n -> p kc n", p=P)
        nc.sync.dma_start(wq_sb[:, :, 0:256], wq_view[:, :, 0:256])
        nc.sync.dma_start(sk_sb[:, KC // 2:], sk_view[:, KC // 2:])
        nc.sync.dma_start(wq_sb[:, :, 256:512], wq_view[:, :, 256:512])

        bqp_sb = persist.tile([P, FC], F32)      # per-partition view for QT bias
        nc.sync.dma_start(bqp_sb[:], bq.rearrange("(c p) -> p c", p=P))
        bqr_sb = persist.tile([1, NV], F32)      # bias as a single row
        nc.sync.dma_start(bqr_sb[:], bq[None, :])
        # bias replicated across partitions, for the free-dim bias add on K/V
        bqb_sb = persist.tile([P, NV], F32)
        nc.gpsimd.partition_broadcast(bqb_sb[:], bqr_sb[:])

        ident_sb = persist.tile([P, P], F16)
        make_identity(nc, ident_sb[:])

        nc.sync.dma_start(wq_sb[:, :, 512:NV], wq_view[:, :, 512:NV])
        sv_sb = persist.tile([P, KC, S_LOC], F16)
        sv_view = svT.rearrange("(kc p) m -> p kc m", p=P)
        nc.sync.dma_start(sv_sb[:, 0:KC // 2], sv_view[:, 0:KC // 2])
        nc.sync.dma_start(sv_sb[:, KC // 2:], sv_view[:, KC // 2:])
        sq_sb = persist.tile([P, KC, S_LOC], F16)
        nc.sync.dma_start(sq_sb[:], sqT.rearrange("(kc p) m -> p kc m", p=P))
        ow_sb = persist.tile([P, KC, NV], F16)   # out_w + ob/16 (host-folded)
        ow_view = owT.rearrange("(kc p) n -> p kc n", p=P)
        nc.sync.dma_start(ow_sb[:, 0:KC // 2], ow_view[:, 0:KC // 2])
        nc.sync.dma_start(ow_sb[:, KC // 2:], ow_view[:, KC // 2:])

        q_sb = persist.tile([P, FC, S_LOC], F16)  # Q^T: feature on partition
        k_sb = persist.tile([P, MC, NV], F16)     # K natural: token on partition
        v_sb = persist.tile([P, MC, NV], F16)
        m_sb = persist.tile([P, NV], F16)         # local K^T V / 8, block-diag
        mr_sb = persist.tile([P, NV], F16)        # after AllReduce

        # ---- K, V projections in natural layout (token on partition) ----
        #   K[m, n] = sum_k Sk[m, k] Wq[n, k] + bq[n]
        #   lhsT = SkT chunk (k on part, token free) ; rhs = WqT chunk
        #   bias (varies along the free dim) folds into the PSUM evacuation
        #   (DVE: GPSIMD cannot read PSUM, ACT cannot add a free-dim bias)
        # half-outer iteration: all of Wq's first column half is consumed
        # before the second half's DMA needs to have landed
        for src, dst in ((sk_sb, k_sb), (sv_sb, v_sb)):
            for half in range(2):
                nsl = slice(half * 512, (half + 1) * 512)
                for mc in range(MC):
                    ps = psproj.tile([P, 512], F32, tag="proj")
                    # first output in N=256 column groups: the second group
                    # waits on Wq cols 256:512 while the first streams
                    nq = 2 if (src is sk_sb and half == 0 and mc == 0) else 1
                    for q in range(nq):
                        w = 512 // nq
                        psl = slice(q * w, (q + 1) * w)
                        qsl = slice(half * 512 + q * w, half * 512 + (q + 1) * w)
                        for kc in range(KC):
                            nc.tensor.matmul(
                                ps[:, psl],
                                (src[:, kc, mc * P:(mc + 1) * P]),
                                (wq_sb[:, kc, qsl]),
                                start=(kc == 0), stop=(kc == KC - 1))
                    nc.vector.tensor_tensor(
                        dst[:, mc, nsl], ps[:], bqb_sb[:, nsl], ALU.add)

        # ---- local M = K^T V / 8 per feature chunk ----------------------
        # feature chunk fc holds heads 2fc, 2fc+1 in its 64x64 diagonal
        # blocks.  fp16 runs full rate at N=128, so compute only the fc x fc
        # 128x128 product (both diagonal 64x64 blocks live inside it).
        nc.vector.memset(m_sb[:], 0.0)
        with tc.tile_pool(name="psm", bufs=2, space="PSUM") as psm:
            for fc in range(FC):
                ps = psm.tile([P, P], F32, tag="m")
                for mc in range(MC):
                    nc.tensor.matmul(
                        ps[:],
                        (k_sb[:, mc, fc * P:(fc + 1) * P]),
                        (v_sb[:, mc, fc * P:(fc + 1) * P]),
                        start=(mc == 0), stop=(mc == MC - 1))
                # DVE, not ACT: the ACT queue is busy with Q-projection
                # bias evacuations, and M gates the collective
                nc.vector.tensor_scalar_mul(
                    m_sb[0:64, fc * P:fc * P + 64],
                    ps[0:64, 0:64], SCALE)
                nc.vector.tensor_scalar_mul(
                    m_sb[64:128, fc * P + 64:fc * P + 128],
                    ps[64:128, 64:128], SCALE)

        # ---- AllReduce M within each batch group (256 KB fp16) ----------
        m_in = dram.tile([P, NV], F16)
        m_out = dram.tile([P, NV], F16)
        nc.sync.dma_start(m_in[:], m_sb[:])
        if collective:
            nc.gpsimd.collective_compute(
                "AllReduce", ALU.add,
                replica_groups=REPLICA_GROUPS,
                ins=[m_in.opt()], outs=[m_out.opt()])
        else:  # single-core perf-model variant: same traffic, no collective
            nc.sync.dma_start(m_out[:], m_in[:])
        nc.sync.dma_start(mr_sb[:], m_out[:])

        # ---- Q projection, transposed layout (feature on partition) -----
        # overlaps with the collective above (no data dependence)
        for fc in range(FC):
            ps = psproj.tile([P, 512], F32, tag="proj")
            for kc in range(KC):
                nc.tensor.matmul(
                    ps[:],
                    (wq_sb[:, kc, fc * P:(fc + 1) * P]),
                    (sq_sb[:, kc, :]),
                    start=(kc == 0), stop=(kc == KC - 1))
            nc.scalar.add(q_sb[:, fc, :], ps[:], bqp_sb[:, fc:fc + 1])

        # ---- Y = Q M (block-diag), segmented softmax, transpose, out-proj
        # pipelined per token chunk with the PE one chunk ahead (see module
        # docstring).  Softmax is split into head-halves; half 1's
        # normalize-mult runs on the otherwise idle GPSIMD.
        x_sb = persist.tile([P, MC, NV], F16)       # softmax output, natural
        xT_sb = persist.tile([P, KC, S_LOC], F16)   # X^T: feature on partition
        z_sb = persist.tile([P, MC, NV], F16)       # output, natural layout
        zv = z.rearrange("(mc p) n -> p mc n", p=P)

        with tc.tile_pool(name="psy", bufs=2, space="PSUM") as psy:

            def emit_y(mc):
                # explicit alternating tags: two distinct PSUM regions so
                # Y(mc+1) never serializes against softmax(mc)'s reads
                yps = psy.tile([P, NV], F32, tag=f"y{mc % 2}", bufs=1)
                for fc in range(FC):
                    nc.tensor.matmul(
                        yps[:, fc * P:(fc + 1) * P],
                        q_sb[:, fc, mc * P:(mc + 1) * P],
                        mr_sb[:, fc * P:(fc + 1) * P],
                        start=True, stop=True)
                return yps

            def emit_softmax(mc, yps):
                y3 = yps.rearrange("p (h d) -> p h d", d=DH)
                x3 = x_sb[:, mc, :].rearrange("p (h d) -> p h d", d=DH)
                HH = H // 2
                for hh in range(2):
                    hsl = slice(hh * HH, (hh + 1) * HH)
                    esl = slice(hh * 512, (hh + 1) * 512)
                    nmx = scratch.tile([P, HH], F32, tag=f"nmx{hh}")
                    nc.vector.reduce_max(nmx[:], y3[:, hsl],
                                         axis=mybir.AxisListType.X,
                                         negate=True)
                    e_sb = scratch.tile([P, 512], F16, tag=f"e{hh}")
                    e3 = e_sb.rearrange("p (h d) -> p h d", d=DH)
                    nc.vector.tensor_tensor(
                        e3, y3[:, hsl],
                        nmx[:, :, None].to_broadcast((P, HH, DH)), ALU.add)
                    nc.scalar.activation(e_sb[:], e_sb[:], AF.Exp)
                    sm = scratch.tile([P, HH], F32, tag=f"sm{hh}")
                    nc.vector.reduce_sum(sm[:], e3, axis=mybir.AxisListType.X)
                    rc = scratch.tile([P, HH], F32, tag=f"rc{hh}")
                    nc.vector.reciprocal(rc[:], sm[:])
                    rcb = rc[:, :, None].to_broadcast((P, HH, DH))
                    # half 0's mult on GPSIMD (frees the DVE to start half
                    # 1's chain); half 1's on DVE right after its reciprocal
                    if hh == 0:
                        nc.gpsimd.tensor_tensor(x3[:, hsl], e3, rcb, ALU.mult)
                    else:
                        nc.vector.tensor_tensor(x3[:, hsl], e3, rcb, ALU.mult)

            def emit_to(mc):
                # transpose token chunk mc back to feature-on-partition, then
                # project.  th=0 covers heads 0-7 (gated by the GPSIMD mult),
                # th=1 heads 8-15 (gated by the DVE mult).
                for th in range(2):
                    pst = psy.tile([P, 512], F16, tag="tp", bufs=2)
                    for f in range(4):
                        fc = th * 4 + f
                        nc.tensor.transpose(
                            pst[:, f * P:(f + 1) * P],
                            x_sb[:, mc, fc * P:(fc + 1) * P], ident_sb[:])
                    nc.scalar.copy(
                        xT_sb[:, th * 4:(th + 1) * 4, mc * P:(mc + 1) * P],
                        pst.rearrange("p (fc m) -> p fc m", m=P))
                #   z[m, n] = sum_k X[m, k] (out_w[n, k] + ob[n]/16)
                #   lhsT = X^T chunk (feat on part, token free); rhs = owT
                for half in range(2):
                    nsl = slice(half * 512, (half + 1) * 512)
                    ps = psproj.tile([P, 512], F32, tag="proj")
                    for kc in range(KC):
                        nc.tensor.matmul(
                            ps[:],
                            (xT_sb[:, kc, mc * P:(mc + 1) * P]),
                            (ow_sb[:, kc, nsl]),
                            start=(kc == 0), stop=(kc == KC - 1))
                    nc.scalar.copy(z_sb[:, mc, nsl], ps[:])
                    nc.sync.dma_start(zv[:, mc, nsl], z_sb[:, mc, nsl])

            for mc in range(MC):
                yps = emit_y(mc)
                emit_softmax(mc, yps)
                if mc >= 1:
                    emit_to(mc - 1)
            emit_to(MC - 1)


_NC_CACHE = None


def _get_nc():
    global _NC_CACHE
    if _NC_CACHE is None:
        _NC_CACHE = build_nc()
    return _NC_CACHE


def make_in_maps(Sq, Sk, Sv, Wq_w, Wq_b, out_w, out_b):
    wqT = np.ascontiguousarray(
        np.asarray(Wq_w, dtype=np.float32).T.astype(np.float16))
    # output bias folded into the output weights: softmax rows sum to
    # exactly 1 per head and there are 16 heads, so X @ (ow + ob/16).T
    # == X @ ow.T + ob
    ow_f = np.asarray(out_w, dtype=np.float32) + \
        np.asarray(out_b, dtype=np.float32)[:, None] / np.float32(H)
    owT = np.ascontiguousarray(ow_f.T.astype(np.float16))
    bq = np.asarray(Wq_b, dtype=np.float32)
    in_maps = []
    for c in range(NCORES):
        b, q = c // 4, c % 4
        rows = slice(q * S_LOC, (q + 1) * S_LOC)
        in_maps.append({
            "sqT": np.ascontiguousarray(
                np.asarray(Sq[b, rows], np.float32).T.astype(np.float16)),
            "skT": np.ascontiguousarray(
                np.asarray(Sk[b, rows], np.float32).T.astype(np.float16)),
            "svT": np.ascontiguousarray(
                np.asarray(Sv[b, rows], np.float32).T.astype(np.float16)),
            "wqT": wqT, "owT": owT, "bq": bq,
        })
    return in_maps


def gather_output(results):
    out = np.empty((B, S, NV), dtype=np.float32)
    for c in range(NCORES):
        b, q = c // 4, c % 4
        out[b, q * S_LOC:(q + 1) * S_LOC, :] = results[c]["z"].astype(
            np.float32)
    return out


def kernel(Sq, Sk, Sv, Wq_w, Wq_b, out_w, out_b, **_unused):
    nc = _get_nc()
    in_maps = make_in_maps(Sq, Sk, Sv, Wq_w, Wq_b, out_w, out_b)
    res = run_bass_kernel_spmd(nc, in_maps, core_ids=list(range(NCORES)))
    return gather_output(res.results)
